# revision 25
# baseline (speedup 1.0000x reference)
"""Multi-head causal attention (B=2, S=2048, D=1024, H=16, DK=DV=64) on 8 Trainium2
NeuronCores.

Sharding: 2-way batch x 4-way head-group. Core i handles batch i//4 and heads
[4*(i%4), 4*(i%4)+4). Each core projects q/k/v for its head group, runs causal
attention, and computes a partial output projection through its row-block of Wo.
The 4 partial outputs per batch are summed on the host (the all-reduce of the
row-sharded Wo output).

All matmul operands are bf16 (psum accumulation stays fp32): every 128-column
stationary is FWL-eligible so LDWEIGHTS hides behind the previous matmul and the
PE stays densely busy (HAM un-throttled at 2.4GHz). q/k live as [dk, s] per head
so scores come out transposed ([s_k, s_q]); v is projected in natural [s_k, dv]
orientation directly (stationary = x^T tile, moving = Wv), then cast into padded
[v | ones | 0] 128-column stationaries - the ones column yields the softmax
denominator as row 64 of the attn@v psum for free.

The whole kernel pipelines per 512-row chunk: project v/k/q for chunk c, then
run chunk c's attention, with chunk c-1's normalize/output-projection emitted
in between so every engine always has independent work queued. Attention runs
two heads at a time: both heads' score matmuls target one [128, 1024] psum tile
and a single wide exp covers both; the attn@v matmuls trail the scores by LAG
tiles so the PE never waits on the scalar engine's exp. Scores and attn@v only
stream the causally-valid column range (triangle trim). Softmax denominators
collect in SBUF rows 32c+h (one DVE reciprocal per chunk covers 4 heads), a
rank-1 ones-outer-product matmul broadcasts each head's reciprocal row, and
gpsimd multiplies the bf16 numerators in place; the output is returned in bf16
and summed on the host in fp64. DMA queues are specialized (x/stg/out on sync
+ gpsimd, weights + den rows on scalar) so no queue stalls another's consumer.
"""
import sys

sys.path.insert(0, "/opt/trn_rl_repo")
import numpy as np

B, S, D = 2, 2048, 1024
H, DK, DV = 16, 64, 64
NCORES = 8
HG = 4          # head-group cores per batch
HPC = H // HG   # heads per core
HDC = HPC * DK  # 256 projection cols per core
P = 128         # partitions
CH = 512        # q-chunk size
XC = 1024       # x-stream DMA chunk
VW = 128        # padded v-stationary width per head: [v(64) | ones | zeros]
LAG = 2         # attn@v trails scores by this many k-tiles


def build(nc, tile, mybir, s=S, d=D):
    F32R = mybir.dt.float32r
    F32 = mybir.dt.float32
    BF16 = mybir.dt.bfloat16
    Exp = mybir.ActivationFunctionType.Exp
    xc = min(XC, s)    # x DMA chunk
    nch = s // CH      # q-chunks
    nst = s // P       # s-tiles (also k-tiles)
    nd = d // P        # d-tiles
    nxc = s // xc      # x DMA chunks
    nm = HDC // P      # head-pair tiles
    spx = xc // CH     # q-chunks per x chunk

    xqT = nc.dram_tensor("xqT", [d, s], BF16, kind="ExternalInput").ap()
    xkT = nc.dram_tensor("xkT", [d, s], BF16, kind="ExternalInput").ap()
    xvT = nc.dram_tensor("xvT", [d, s], BF16, kind="ExternalInput").ap()
    wqkv = nc.dram_tensor("wqkv", [d, 3 * HDC], BF16, kind="ExternalInput").ap()
    wo = nc.dram_tensor("wo", [HDC, d], BF16, kind="ExternalInput").ap()
    maskA = nc.dram_tensor("maskA", [P, P], BF16, kind="ExternalInput").ap()
    vinit = nc.dram_tensor("vinit", [P, nst * HPC * VW], BF16,
                           kind="ExternalInput").ap()
    onesf = nc.dram_tensor("onesf", [1, P], F32R, kind="ExternalInput").ap()
    out = nc.dram_tensor("out", [s, d], BF16, kind="ExternalOutput").ap()

    with tile.TileContext(nc) as tc:
        from contextlib import ExitStack
        with ExitStack() as ctx:
            wp = ctx.enter_context(tc.tile_pool(name="wp", bufs=1))
            xp = ctx.enter_context(tc.tile_pool(name="xp", bufs=24))
            per = ctx.enter_context(tc.tile_pool(name="per", bufs=1))
            ep = ctx.enter_context(tc.tile_pool(name="ep", bufs=6))
            sp = ctx.enter_context(tc.tile_pool(name="sp", bufs=2))
            obp = ctx.enter_context(tc.tile_pool(name="obp", bufs=3))
            scp = ctx.enter_context(tc.tile_pool(name="scp", bufs=2, space="PSUM"))
            ovp = ctx.enter_context(tc.tile_pool(name="ovp", bufs=4, space="PSUM"))

            # --- persistent tiles ---
            wqkv_t = [wp.tile([P, 3 * HDC], BF16, name=f"wqkv{i}")
                      for i in range(nd)]
            wq_t = [wqkv_t[i][:, 0:HDC] for i in range(nd)]
            wk_t = [wqkv_t[i][:, HDC:2 * HDC] for i in range(nd)]
            wv_t = [wqkv_t[i][:, 2 * HDC:3 * HDC] for i in range(nd)]
            wo_t = [wp.tile([P, d], BF16, name=f"wo{i}") for i in range(nm)]
            mA = wp.tile([P, P], BF16, name="mA")
            onf = wp.tile([1, P], F32R, name="onf")
            qT = [per.tile([P, s], BF16, name=f"qT{m}") for m in range(nm)]
            kTt = [per.tile([P, s], BF16, name=f"kT{m}") for m in range(nm)]
            oT = [per.tile([P, s], BF16, name=f"oT{m}") for m in range(nm)]
            vaug = per.tile([P, nst * HPC * VW], BF16, name="vaug")
            den = per.tile([P, CH], F32, name="den")
            rec = per.tile([P, CH], F32R, name="rec")

            # --- initial loads: first-chunk halves first so vproj(0)
            # starts after ~1/6 of the x bytes are in ---
            xt = {}  # (stream, sc, dd) -> tile
            rr = [0]
            def issue_half(stream, xsrc, sc, hf, engs):
                for dd in range(nd):
                    key = (stream, sc, dd)
                    if key not in xt:
                        xt[key] = xp.tile([P, xc], BF16, name="xt", tag="xt")
                    eng = engs[rr[0] % len(engs)]
                    rr[0] += 1
                    eng.dma_start(
                        xt[key][:, hf:hf + CH],
                        xsrc[dd * P:(dd + 1) * P,
                             sc * xc + hf:sc * xc + hf + CH])

            q3 = (nc.sync, nc.gpsimd, nc.scalar)
            q2 = (nc.sync, nc.gpsimd)
            for i in range(nd):
                q3[i % 3].dma_start(wqkv_t[i][:], wqkv[i * P:(i + 1) * P, :])
            nc.scalar.dma_start(mA[:], maskA[:, :])
            issue_half(2, xqT, 0, 0, q3)
            issue_half(1, xkT, 0, 0, q3)
            issue_half(0, xvT, 0, 0, q3)
            issue_half(0, xvT, 0, CH, q3)
            issue_half(1, xkT, 0, CH, q3)
            issue_half(2, xqT, 0, CH, q3)
            nc.scalar.dma_start(onf[:], onesf[:, :])
            for i in range(nm):
                nc.scalar.dma_start(wo_t[i][:], wo[i * P:(i + 1) * P, :])
            half = nst * HPC * VW // 2
            nc.sync.dma_start(vaug[:, 0:half], vinit[:, 0:half])
            nc.gpsimd.dma_start(vaug[:, half:], vinit[:, half:])

            def vproject(c):
                """vaug[st-block of chunk c] = v natural [s_k, dv] per head."""
                sc, hf = c // spx, (c % spx) * CH
                pp = scp.tile([P, 2 * CH], F32, name="pv", tag="sc")
                for j in range(CH // P):
                    for dd in range(nd):
                        nc.tensor.matmul(
                            pp[:, j * HDC:(j + 1) * HDC],
                            xt[(0, sc, dd)][:, hf + j * P:hf + (j + 1) * P],
                            wv_t[dd][:],
                            start=(dd == 0), stop=(dd == nd - 1))
                for j in range(CH // P):
                    st = c * (CH // P) + j
                    base = st * HPC * VW
                    dst = vaug[:, base:base + HPC * VW].rearrange(
                        "p (h x) -> p h x", x=VW)[:, :, 0:DV]
                    src = pp[:, j * HDC:(j + 1) * HDC].rearrange(
                        "p (h x) -> p h x", x=DV)
                    nc.vector.tensor_copy(dst, src)

            def project(stream, w_t, dstT, c):
                """dstT[m][:, c*CH:(c+1)*CH] for both m from one psum tile."""
                sc, hf = c // spx, (c % spx) * CH
                pp = scp.tile([P, 2 * CH], F32, name="pbig", tag="sc")
                for m in range(nm):
                    for dd in range(nd):
                        nc.tensor.matmul(
                            pp[:, m * CH:(m + 1) * CH],
                            w_t[dd][:, m * P:(m + 1) * P],
                            xt[(stream, sc, dd)][:, hf:hf + CH],
                            start=(dd == 0), stop=(dd == nd - 1))
                for m in range(nm):
                    dsl = dstT[m][:, c * CH:(c + 1) * CH]
                    if m % 2 == 0:
                        nc.scalar.copy(dsl, pp[:, m * CH:(m + 1) * CH])
                    else:
                        nc.vector.tensor_copy(dsl, pp[:, m * CH:(m + 1) * CH])

            def attention_pair(hp, c):
                """Heads hp, hp+1 (one m-tile) over chunk c, ov lagged."""
                mi = hp // 2
                nt = 4 * c + 4  # k-tiles for this chunk
                ov = [ovp.tile([P, CH], F32, name=f"ov{j}", tag="ov")
                      for j in range(2)]
                pend = []

                def emit_ov(t, ex, lo):
                    for j in range(2):
                        vb = t * HPC * VW + (hp + j) * VW
                        nc.tensor.matmul(
                            ov[j][:, lo:CH],
                            vaug[:, vb:vb + VW],
                            ex[:, j * CH + lo:(j + 1) * CH],
                            start=(t == 0), stop=(t == nt - 1))

                for t in range(nt):
                    r = t - 4 * c
                    lo = max(r, 0) * P  # first valid column in the chunk
                    sc_t = scp.tile([P, 2 * CH], F32, name="scp", tag="sc")
                    for j in range(2):
                        nc.tensor.matmul(
                            sc_t[:, j * CH + lo:(j + 1) * CH],
                            kTt[mi][j * DK:(j + 1) * DK, t * P:(t + 1) * P],
                            qT[mi][j * DK:(j + 1) * DK,
                                   c * CH + lo:(c + 1) * CH],
                            start=True, stop=True)
                    ex = ep.tile([P, 2 * CH], BF16, name="ex", tag="ex")
                    sview = sc_t[:].rearrange("p (g x) -> p g x", x=CH)
                    eview = ex[:].rearrange("p (g x) -> p g x", x=CH)
                    nc.scalar.activation(eview[:, :, lo:CH],
                                         sview[:, :, lo:CH], Exp)
                    if r >= 0:
                        for j in range(2):
                            nc.vector.tensor_mul(
                                ex[:, j * CH + lo:j * CH + lo + P],
                                ex[:, j * CH + lo:j * CH + lo + P], mA[:])
                    pend.append((t, ex, lo))
                    if len(pend) > LAG:
                        emit_ov(*pend.pop(0))
                while pend:
                    emit_ov(*pend.pop(0))
                # numerator rows 0:64 -> oT (unnormalized, bf16);
                # denominator row 64 -> denw col 4c+h (partition-spread)
                tail_pair = c == nch - 1 and hp == 2
                for j in range(2):
                    h, ri = hp + j, j * DK
                    dsl = oT[mi][ri:ri + DK, c * CH:(c + 1) * CH]
                    dstg = sp.tile([1, CH], F32, name="dstg", tag="dstg", bufs=4)
                    if tail_pair:
                        nc.scalar.copy(dsl, ov[j][0:DV, :])
                        nc.scalar.copy(dstg[:], ov[j][DV:DV + 1, :])
                    else:
                        nc.vector.tensor_copy(dsl, ov[j][0:DV, :])
                        nc.vector.tensor_copy(dstg[:], ov[j][DV:DV + 1, :])
                    nc.scalar.dma_start(den[32 * c + h:32 * c + h + 1, :],
                                         dstg[:])

            def normalize_pair(c, hp):
                """Scale oT rows of heads hp,hp+1 of chunk c by 1/denominator."""
                mi = hp // 2
                last = c == nch - 1
                def recip(n_):
                    r0 = 32 * c
                    with nc.allow_low_precision(reason="softmax denom recip"):
                        nc.vector.reciprocal(rec[r0:r0 + n_, :],
                                             den[r0:r0 + n_, :])
                if hp == 0 and not last:
                    recip(HPC)
                if last:
                    recip(2 if hp == 0 else HPC)
                rb = scp.tile([P, 2 * CH], F32, name="rb", tag="sc")
                for j in range(2):
                    h = hp + j
                    stg = sp.tile([1, CH], F32R, name="stg", tag="stg", bufs=4)
                    nc.sync.dma_start(stg[:],
                                      rec[32 * c + h:32 * c + h + 1, :])
                    nc.tensor.matmul(rb[:, j * CH:(j + 1) * CH],
                                     onf[0:1, :], stg[:],
                                     start=True, stop=True)
                if last and hp == 2:
                    for j in range(2):
                        ri = j * DK
                        sl = oT[mi][ri:ri + DK, c * CH:(c + 1) * CH]
                        nc.vector.tensor_mul(
                            sl, sl, rb[ri:ri + DK, j * CH:(j + 1) * CH])
                else:
                    recT = sp.tile([P, 2 * CH], BF16, name="recT", tag="recT",
                                   bufs=2)
                    nc.vector.tensor_copy(recT[:], rb[:])
                    for j in range(2):
                        ri = j * DK
                        sl = oT[mi][ri:ri + DK, c * CH:(c + 1) * CH]
                        nc.gpsimd.tensor_mul(
                            sl, sl, recT[ri:ri + DK, j * CH:(j + 1) * CH])

            def oproj(st):
                pp = scp.tile([P, 2 * CH], F32, name="pout", tag="sc")
                for n in range(d // 512):
                    for m in range(nm):
                        nc.tensor.matmul(pp[:, n * 512:(n + 1) * 512],
                                         oT[m][:, st * P:(st + 1) * P],
                                         wo_t[m][:, n * 512:(n + 1) * 512],
                                         start=(m == 0), stop=(m == nm - 1))
                ob = obp.tile([P, d], BF16, name="ob", tag="ob")
                if st % 2 == 0:
                    nc.scalar.copy(ob[:], pp[:])
                else:
                    nc.vector.tensor_copy(ob[:], pp[:])
                nc.sync.dma_start(out[st * P:(st + 1) * P, :], ob[:])

            # --- per-chunk pipeline ---
            def proj_chunk(c):
                project(2, wq_t, qT, c)
                project(1, wk_t, kTt, c)
                vproject(c)

            prev = None
            proj_chunk(0)
            for c in range(nch):
                attention_pair(0, c)
                if c == 1:
                    issue_half(0, xvT, 1, 0, q2)
                    issue_half(0, xvT, 1, CH, q2)
                if prev is not None:
                    normalize_pair(prev, 0)
                    normalize_pair(prev, 2)
                attention_pair(2, c)
                if c == 1:
                    issue_half(1, xkT, 1, 0, q2)
                    issue_half(2, xqT, 1, 0, q2)
                    issue_half(1, xkT, 1, CH, q2)
                    issue_half(2, xqT, 1, CH, q2)
                if c + 1 < nch:
                    proj_chunk(c + 1)
                if c == nch - 1:
                    normalize_pair(c, 0)
                if prev is not None:
                    for st in range(prev * CH // P, (prev + 1) * CH // P):
                        oproj(st)
                prev = c
            normalize_pair(prev, 2)
            for st in range(prev * CH // P, (prev + 1) * CH // P):
                oproj(st)
    nc.compile()
    return nc


_NC_CACHE = {}
LAST_RESULT = None


def _get_nc(s=S, d=D):
    key = (s, d)
    if key not in _NC_CACHE:
        import concourse.tile as tile
        import concourse.mybir as mybir
        from concourse import bacc
        nc = bacc.Bacc("TRN2", target_bir_lowering=False, num_devices=NCORES)
        _NC_CACHE[key] = build(nc, tile, mybir, s=s, d=d)
    return _NC_CACHE[key]


def make_consts():
    import ml_dtypes
    i = np.arange(P)[:, None]
    j = np.arange(P)[None, :]
    maskA = (j >= i).astype(ml_dtypes.bfloat16)
    nst = S // P
    vinit = np.zeros((P, nst * HPC * VW), dtype=ml_dtypes.bfloat16)
    vinit[:, DV::VW] = 1
    onesf = np.ones((1, P), dtype=np.float32)
    return maskA, vinit, onesf


def kernel(Q, K, V, Wq, Wk, Wv, Wo):
    import ml_dtypes
    from concourse.bass_utils import run_bass_kernel_spmd

    BF = ml_dtypes.bfloat16
    Q = np.asarray(Q, dtype=np.float32)
    K = np.asarray(K, dtype=np.float32)
    V = np.asarray(V, dtype=np.float32)
    Wq = np.asarray(Wq, dtype=np.float32) * np.float32(1.0 / np.sqrt(DK))
    Wk = np.asarray(Wk, dtype=np.float32)
    Wv = np.asarray(Wv, dtype=np.float32)
    Wo = np.asarray(Wo, dtype=np.float32)

    QT = [np.ascontiguousarray(Q[b].T).astype(BF) for b in range(B)]
    KT = [np.ascontiguousarray(K[b].T).astype(BF) for b in range(B)]
    VT = [np.ascontiguousarray(V[b].T).astype(BF) for b in range(B)]
    maskA, vinit, onesf = make_consts()

    in_maps = []
    for core in range(NCORES):
        b, g = core // HG, core % HG
        cs = slice(g * HDC, (g + 1) * HDC)
        in_maps.append({
            "xqT": QT[b], "xkT": KT[b], "xvT": VT[b],
            "wqkv": np.ascontiguousarray(
                np.concatenate([Wq[:, cs], Wk[:, cs], Wv[:, cs]],
                               axis=1)).astype(BF),
            "wo": np.ascontiguousarray(Wo[cs, :]).astype(BF),
            "maskA": maskA, "vinit": vinit, "onesf": onesf,
        })

    nc = _get_nc()
    res = run_bass_kernel_spmd(nc, in_maps, core_ids=list(range(NCORES)))
    global LAST_RESULT
    LAST_RESULT = res

    acc = np.zeros((B, S, D), dtype=np.float64)
    for core in range(NCORES):
        acc[core // HG] += res.results[core]["out"].astype(np.float64)
    return acc.astype(np.float32)


# revision 27
# speedup vs baseline: 1.0017x; 1.0017x over previous
"""Multi-head causal attention (B=2, S=2048, D=1024, H=16, DK=DV=64) on 8 Trainium2
NeuronCores.

Sharding: 2-way batch x 4-way head-group. Core i handles batch i//4 and heads
[4*(i%4), 4*(i%4)+4). Each core projects q/k/v for its head group, runs causal
attention, and computes a partial output projection through its row-block of Wo.
The 4 partial outputs per batch are summed on the host (the all-reduce of the
row-sharded Wo output).

All matmul operands are bf16 (psum accumulation stays fp32): every 128-column
stationary is FWL-eligible so LDWEIGHTS hides behind the previous matmul and the
PE stays densely busy (HAM un-throttled at 2.4GHz). q/k live as [dk, s] per head
so scores come out transposed ([s_k, s_q]); v is projected in natural [s_k, dv]
orientation directly (stationary = x^T tile, moving = Wv), then cast into padded
[v | ones | 0] 128-column stationaries - the ones column yields the softmax
denominator as row 64 of the attn@v psum for free.

The whole kernel pipelines per 512-row chunk: project v/k/q for chunk c, then
run chunk c's attention, with chunk c-1's normalize/output-projection emitted
in between so every engine always has independent work queued. Attention runs
two heads at a time: both heads' score matmuls target one [128, 1024] psum tile
and a single wide exp covers both; the attn@v matmuls trail the scores by LAG
tiles so the PE never waits on the scalar engine's exp. Scores and attn@v only
stream the causally-valid column range (triangle trim). Softmax denominators
collect in SBUF rows 32c+h (one DVE reciprocal per chunk covers 4 heads), a
rank-1 ones-outer-product matmul broadcasts each head's reciprocal row, and
gpsimd multiplies the bf16 numerators in place; the output is returned in bf16
and summed on the host in fp64. DMA queues are specialized (x/stg/out on sync
+ gpsimd, weights + den rows on scalar) so no queue stalls another's consumer.
"""
import sys

sys.path.insert(0, "/opt/trn_rl_repo")
import numpy as np

B, S, D = 2, 2048, 1024
H, DK, DV = 16, 64, 64
NCORES = 8
HG = 4          # head-group cores per batch
HPC = H // HG   # heads per core
HDC = HPC * DK  # 256 projection cols per core
P = 128         # partitions
CH = 512        # q-chunk size
XC = 1024       # x-stream DMA chunk
VW = 128        # padded v-stationary width per head: [v(64) | ones | zeros]
LAG = 2         # attn@v trails scores by this many k-tiles


def build(nc, tile, mybir, s=S, d=D):
    F32R = mybir.dt.float32r
    F32 = mybir.dt.float32
    BF16 = mybir.dt.bfloat16
    Exp = mybir.ActivationFunctionType.Exp
    xc = min(XC, s)    # x DMA chunk
    nch = s // CH      # q-chunks
    nst = s // P       # s-tiles (also k-tiles)
    nd = d // P        # d-tiles
    nxc = s // xc      # x DMA chunks
    nm = HDC // P      # head-pair tiles
    spx = xc // CH     # q-chunks per x chunk

    xqT = nc.dram_tensor("xqT", [d, s], BF16, kind="ExternalInput").ap()
    xkT = nc.dram_tensor("xkT", [d, s], BF16, kind="ExternalInput").ap()
    xvT = nc.dram_tensor("xvT", [d, s], BF16, kind="ExternalInput").ap()
    wqkv = nc.dram_tensor("wqkv", [d, 3 * HDC], BF16, kind="ExternalInput").ap()
    wo = nc.dram_tensor("wo", [HDC, d], BF16, kind="ExternalInput").ap()
    maskA = nc.dram_tensor("maskA", [P, P], BF16, kind="ExternalInput").ap()
    vinit = nc.dram_tensor("vinit", [P, nst * HPC * VW], BF16,
                           kind="ExternalInput").ap()
    onesf = nc.dram_tensor("onesf", [1, P], F32R, kind="ExternalInput").ap()
    out = nc.dram_tensor("out", [s, d], BF16, kind="ExternalOutput").ap()

    with tile.TileContext(nc) as tc:
        from contextlib import ExitStack
        with ExitStack() as ctx:
            wp = ctx.enter_context(tc.tile_pool(name="wp", bufs=1))
            xp = ctx.enter_context(tc.tile_pool(name="xp", bufs=6))
            per = ctx.enter_context(tc.tile_pool(name="per", bufs=1))
            ep = ctx.enter_context(tc.tile_pool(name="ep", bufs=6))
            sp = ctx.enter_context(tc.tile_pool(name="sp", bufs=2))
            obp = ctx.enter_context(tc.tile_pool(name="obp", bufs=3))
            scp = ctx.enter_context(tc.tile_pool(name="scp", bufs=2, space="PSUM"))
            ovp = ctx.enter_context(tc.tile_pool(name="ovp", bufs=4, space="PSUM"))

            # --- persistent tiles ---
            wqkv_t = [wp.tile([P, 3 * HDC], BF16, name=f"wqkv{i}")
                      for i in range(nd)]
            wq_t = [wqkv_t[i][:, 0:HDC] for i in range(nd)]
            wk_t = [wqkv_t[i][:, HDC:2 * HDC] for i in range(nd)]
            wv_t = [wqkv_t[i][:, 2 * HDC:3 * HDC] for i in range(nd)]
            wo_t = [wp.tile([P, d], BF16, name=f"wo{i}") for i in range(nm)]
            mA = wp.tile([P, P], BF16, name="mA")
            onf = wp.tile([1, P], F32R, name="onf")
            qT = [per.tile([P, s], BF16, name=f"qT{m}") for m in range(nm)]
            kTt = [per.tile([P, s], BF16, name=f"kT{m}") for m in range(nm)]
            oT = [per.tile([P, s], BF16, name=f"oT{m}") for m in range(nm)]
            vaug = per.tile([P, nst * HPC * VW], BF16, name="vaug")
            den = per.tile([P, CH], F32, name="den")
            rec = per.tile([P, CH], F32R, name="rec")

            # --- x loads: one DMA per (stream, 512-col half) covers all 8
            # d-tiles (contiguous DRAM rows -> strided src AP), so the three
            # streams load in parallel on the three DMA queues ---
            xt = {}  # (stream, chunk) -> [P, nd*CH] tile, d-tiles side by side
            def issue_xh(stream, xsrc, c, eng):
                t = xp.tile([P, nd * CH], BF16, name="xt", tag="xt")
                xt[(stream, c)] = t
                eng.dma_start(
                    t[:].rearrange("p (dd x) -> p dd x", x=CH),
                    xsrc[:, c * CH:(c + 1) * CH].rearrange(
                        "(dd p) x -> p dd x", p=P))

            q3 = (nc.sync, nc.gpsimd, nc.scalar)
            q2 = (nc.sync, nc.gpsimd)
            issue_xh(2, xqT, 0, nc.sync)
            issue_xh(1, xkT, 0, nc.gpsimd)
            nc.scalar.dma_start(mA[:], maskA[:, :])
            for i in range(nd):
                q3[i % 3].dma_start(wqkv_t[i][:], wqkv[i * P:(i + 1) * P, :])
            issue_xh(0, xvT, 0, nc.scalar)
            issue_xh(2, xqT, 1, nc.sync)
            issue_xh(1, xkT, 1, nc.gpsimd)
            issue_xh(0, xvT, 1, nc.scalar)
            nc.scalar.dma_start(onf[:], onesf[:, :])
            for i in range(nm):
                nc.scalar.dma_start(wo_t[i][:], wo[i * P:(i + 1) * P, :])
            half = nst * HPC * VW // 2
            nc.sync.dma_start(vaug[:, 0:half], vinit[:, 0:half])
            nc.gpsimd.dma_start(vaug[:, half:], vinit[:, half:])

            def vproject(c):
                """vaug[st-block of chunk c] = v natural [s_k, dv] per head."""
                xh = xt[(0, c)]
                pp = scp.tile([P, 2 * CH], F32, name="pv", tag="sc")
                for j in range(CH // P):
                    for dd in range(nd):
                        nc.tensor.matmul(
                            pp[:, j * HDC:(j + 1) * HDC],
                            xh[:, dd * CH + j * P:dd * CH + (j + 1) * P],
                            wv_t[dd][:],
                            start=(dd == 0), stop=(dd == nd - 1))
                for j in range(CH // P):
                    st = c * (CH // P) + j
                    base = st * HPC * VW
                    dst = vaug[:, base:base + HPC * VW].rearrange(
                        "p (h x) -> p h x", x=VW)[:, :, 0:DV]
                    src = pp[:, j * HDC:(j + 1) * HDC].rearrange(
                        "p (h x) -> p h x", x=DV)
                    nc.vector.tensor_copy(dst, src)

            def project(stream, w_t, dstT, c):
                """dstT[m][:, c*CH:(c+1)*CH] for both m from one psum tile."""
                xh = xt[(stream, c)]
                pp = scp.tile([P, 2 * CH], F32, name="pbig", tag="sc")
                for m in range(nm):
                    for dd in range(nd):
                        nc.tensor.matmul(
                            pp[:, m * CH:(m + 1) * CH],
                            w_t[dd][:, m * P:(m + 1) * P],
                            xh[:, dd * CH:(dd + 1) * CH],
                            start=(dd == 0), stop=(dd == nd - 1))
                for m in range(nm):
                    dsl = dstT[m][:, c * CH:(c + 1) * CH]
                    if m % 2 == 0:
                        nc.scalar.copy(dsl, pp[:, m * CH:(m + 1) * CH])
                    else:
                        nc.vector.tensor_copy(dsl, pp[:, m * CH:(m + 1) * CH])

            def attention_pair(hp, c):
                """Heads hp, hp+1 (one m-tile) over chunk c, ov lagged."""
                mi = hp // 2
                nt = 4 * c + 4  # k-tiles for this chunk
                ov = [ovp.tile([P, CH], F32, name=f"ov{j}", tag="ov")
                      for j in range(2)]
                pend = []

                def emit_ov(t, ex, lo):
                    for j in range(2):
                        vb = t * HPC * VW + (hp + j) * VW
                        nc.tensor.matmul(
                            ov[j][:, lo:CH],
                            vaug[:, vb:vb + VW],
                            ex[:, j * CH + lo:(j + 1) * CH],
                            start=(t == 0), stop=(t == nt - 1))

                for t in range(nt):
                    r = t - 4 * c
                    lo = max(r, 0) * P  # first valid column in the chunk
                    sc_t = scp.tile([P, 2 * CH], F32, name="scp", tag="sc")
                    for j in range(2):
                        nc.tensor.matmul(
                            sc_t[:, j * CH + lo:(j + 1) * CH],
                            kTt[mi][j * DK:(j + 1) * DK, t * P:(t + 1) * P],
                            qT[mi][j * DK:(j + 1) * DK,
                                   c * CH + lo:(c + 1) * CH],
                            start=True, stop=True)
                    ex = ep.tile([P, 2 * CH], BF16, name="ex", tag="ex")
                    sview = sc_t[:].rearrange("p (g x) -> p g x", x=CH)
                    eview = ex[:].rearrange("p (g x) -> p g x", x=CH)
                    nc.scalar.activation(eview[:, :, lo:CH],
                                         sview[:, :, lo:CH], Exp)
                    if r >= 0:
                        for j in range(2):
                            nc.vector.tensor_mul(
                                ex[:, j * CH + lo:j * CH + lo + P],
                                ex[:, j * CH + lo:j * CH + lo + P], mA[:])
                    pend.append((t, ex, lo))
                    if len(pend) > LAG:
                        emit_ov(*pend.pop(0))
                while pend:
                    emit_ov(*pend.pop(0))
                # numerator rows 0:64 -> oT (unnormalized, bf16);
                # denominator row 64 -> denw col 4c+h (partition-spread)
                tail_pair = c == nch - 1 and hp == 2
                for j in range(2):
                    h, ri = hp + j, j * DK
                    dsl = oT[mi][ri:ri + DK, c * CH:(c + 1) * CH]
                    dstg = sp.tile([1, CH], F32, name="dstg", tag="dstg", bufs=4)
                    if tail_pair:
                        nc.scalar.copy(dsl, ov[j][0:DV, :])
                        nc.scalar.copy(dstg[:], ov[j][DV:DV + 1, :])
                    else:
                        nc.vector.tensor_copy(dsl, ov[j][0:DV, :])
                        nc.vector.tensor_copy(dstg[:], ov[j][DV:DV + 1, :])
                    nc.scalar.dma_start(den[32 * c + h:32 * c + h + 1, :],
                                         dstg[:])

            def normalize_pair(c, hp):
                """Scale oT rows of heads hp,hp+1 of chunk c by 1/denominator."""
                mi = hp // 2
                last = c == nch - 1
                def recip(n_):
                    r0 = 32 * c
                    with nc.allow_low_precision(reason="softmax denom recip"):
                        nc.vector.reciprocal(rec[r0:r0 + n_, :],
                                             den[r0:r0 + n_, :])
                if hp == 0 and not last:
                    recip(HPC)
                if last:
                    recip(2 if hp == 0 else HPC)
                rb = scp.tile([P, 2 * CH], F32, name="rb", tag="sc")
                for j in range(2):
                    h = hp + j
                    stg = sp.tile([1, CH], F32R, name="stg", tag="stg", bufs=4)
                    nc.sync.dma_start(stg[:],
                                      rec[32 * c + h:32 * c + h + 1, :])
                    nc.tensor.matmul(rb[:, j * CH:(j + 1) * CH],
                                     onf[0:1, :], stg[:],
                                     start=True, stop=True)
                if last and hp == 2:
                    for j in range(2):
                        ri = j * DK
                        sl = oT[mi][ri:ri + DK, c * CH:(c + 1) * CH]
                        nc.vector.tensor_mul(
                            sl, sl, rb[ri:ri + DK, j * CH:(j + 1) * CH])
                else:
                    recT = sp.tile([P, 2 * CH], BF16, name="recT", tag="recT",
                                   bufs=2)
                    nc.vector.tensor_copy(recT[:], rb[:])
                    for j in range(2):
                        ri = j * DK
                        sl = oT[mi][ri:ri + DK, c * CH:(c + 1) * CH]
                        nc.gpsimd.tensor_mul(
                            sl, sl, recT[ri:ri + DK, j * CH:(j + 1) * CH])

            def oproj(st):
                pp = scp.tile([P, 2 * CH], F32, name="pout", tag="sc")
                for n in range(d // 512):
                    for m in range(nm):
                        nc.tensor.matmul(pp[:, n * 512:(n + 1) * 512],
                                         oT[m][:, st * P:(st + 1) * P],
                                         wo_t[m][:, n * 512:(n + 1) * 512],
                                         start=(m == 0), stop=(m == nm - 1))
                ob = obp.tile([P, d], BF16, name="ob", tag="ob")
                if st % 2 == 0:
                    nc.scalar.copy(ob[:], pp[:])
                else:
                    nc.vector.tensor_copy(ob[:], pp[:])
                nc.sync.dma_start(out[st * P:(st + 1) * P, :], ob[:])

            # --- per-chunk pipeline ---
            def proj_chunk(c):
                project(2, wq_t, qT, c)
                project(1, wk_t, kTt, c)
                vproject(c)

            prev = None
            proj_chunk(0)
            for c in range(nch):
                attention_pair(0, c)
                if c == 1:
                    issue_xh(2, xqT, 2, nc.sync)
                    issue_xh(1, xkT, 2, nc.gpsimd)
                if prev is not None:
                    normalize_pair(prev, 0)
                    normalize_pair(prev, 2)
                attention_pair(2, c)
                if c == 1:
                    issue_xh(0, xvT, 2, nc.sync)
                    issue_xh(2, xqT, 3, nc.sync)
                    issue_xh(1, xkT, 3, nc.gpsimd)
                    issue_xh(0, xvT, 3, nc.gpsimd)
                if c + 1 < nch:
                    proj_chunk(c + 1)
                if c == nch - 1:
                    normalize_pair(c, 0)
                if prev is not None:
                    for st in range(prev * CH // P, (prev + 1) * CH // P):
                        oproj(st)
                prev = c
            normalize_pair(prev, 2)
            for st in range(prev * CH // P, (prev + 1) * CH // P):
                oproj(st)
    nc.compile()
    return nc


_NC_CACHE = {}
LAST_RESULT = None


def _get_nc(s=S, d=D):
    key = (s, d)
    if key not in _NC_CACHE:
        import concourse.tile as tile
        import concourse.mybir as mybir
        from concourse import bacc
        nc = bacc.Bacc("TRN2", target_bir_lowering=False, num_devices=NCORES)
        _NC_CACHE[key] = build(nc, tile, mybir, s=s, d=d)
    return _NC_CACHE[key]


def make_consts():
    import ml_dtypes
    i = np.arange(P)[:, None]
    j = np.arange(P)[None, :]
    maskA = (j >= i).astype(ml_dtypes.bfloat16)
    nst = S // P
    vinit = np.zeros((P, nst * HPC * VW), dtype=ml_dtypes.bfloat16)
    vinit[:, DV::VW] = 1
    onesf = np.ones((1, P), dtype=np.float32)
    return maskA, vinit, onesf


def kernel(Q, K, V, Wq, Wk, Wv, Wo):
    import ml_dtypes
    from concourse.bass_utils import run_bass_kernel_spmd

    BF = ml_dtypes.bfloat16
    Q = np.asarray(Q, dtype=np.float32)
    K = np.asarray(K, dtype=np.float32)
    V = np.asarray(V, dtype=np.float32)
    Wq = np.asarray(Wq, dtype=np.float32) * np.float32(1.0 / np.sqrt(DK))
    Wk = np.asarray(Wk, dtype=np.float32)
    Wv = np.asarray(Wv, dtype=np.float32)
    Wo = np.asarray(Wo, dtype=np.float32)

    QT = [np.ascontiguousarray(Q[b].T).astype(BF) for b in range(B)]
    KT = [np.ascontiguousarray(K[b].T).astype(BF) for b in range(B)]
    VT = [np.ascontiguousarray(V[b].T).astype(BF) for b in range(B)]
    maskA, vinit, onesf = make_consts()

    in_maps = []
    for core in range(NCORES):
        b, g = core // HG, core % HG
        cs = slice(g * HDC, (g + 1) * HDC)
        in_maps.append({
            "xqT": QT[b], "xkT": KT[b], "xvT": VT[b],
            "wqkv": np.ascontiguousarray(
                np.concatenate([Wq[:, cs], Wk[:, cs], Wv[:, cs]],
                               axis=1)).astype(BF),
            "wo": np.ascontiguousarray(Wo[cs, :]).astype(BF),
            "maskA": maskA, "vinit": vinit, "onesf": onesf,
        })

    nc = _get_nc()
    res = run_bass_kernel_spmd(nc, in_maps, core_ids=list(range(NCORES)))
    global LAST_RESULT
    LAST_RESULT = res

    acc = np.zeros((B, S, D), dtype=np.float64)
    for core in range(NCORES):
        acc[core // HG] += res.results[core]["out"].astype(np.float64)
    return acc.astype(np.float32)


# revision 28
# speedup vs baseline: 1.0034x; 1.0017x over previous
"""Multi-head causal attention (B=2, S=2048, D=1024, H=16, DK=DV=64) on 8 Trainium2
NeuronCores.

Sharding: 2-way batch x 4-way head-group. Core i handles batch i//4 and heads
[4*(i%4), 4*(i%4)+4). Each core projects q/k/v for its head group, runs causal
attention, and computes a partial output projection through its row-block of Wo.
The 4 partial outputs per batch are summed on the host (the all-reduce of the
row-sharded Wo output).

All matmul operands are bf16 (psum accumulation stays fp32): every 128-column
stationary is FWL-eligible so LDWEIGHTS hides behind the previous matmul and the
PE stays densely busy (HAM un-throttled at 2.4GHz). q/k live as [dk, s] per head
so scores come out transposed ([s_k, s_q]); v is projected in natural [s_k, dv]
orientation directly (stationary = x^T tile, moving = Wv), then cast into padded
[v | ones | 0] 128-column stationaries - the ones column yields the softmax
denominator as row 64 of the attn@v psum for free.

The whole kernel pipelines per 512-row chunk: project v/k/q for chunk c, then
run chunk c's attention, with chunk c-1's normalize/output-projection emitted
in between so every engine always has independent work queued. Attention runs
two heads at a time: both heads' score matmuls target one [128, 1024] psum tile
and a single wide exp covers both; the attn@v matmuls trail the scores by LAG
tiles so the PE never waits on the scalar engine's exp. Scores and attn@v only
stream the causally-valid column range (triangle trim). Softmax denominators
collect in SBUF rows 32c+h (one DVE reciprocal per chunk covers 4 heads), a
rank-1 ones-outer-product matmul broadcasts each head's reciprocal row, and
gpsimd multiplies the bf16 numerators in place; the output is returned in bf16
and summed on the host in fp64. DMA queues are specialized (x/stg/out on sync
+ gpsimd, weights + den rows on scalar) so no queue stalls another's consumer.
"""
import sys

sys.path.insert(0, "/opt/trn_rl_repo")
import numpy as np

B, S, D = 2, 2048, 1024
H, DK, DV = 16, 64, 64
NCORES = 8
HG = 4          # head-group cores per batch
HPC = H // HG   # heads per core
HDC = HPC * DK  # 256 projection cols per core
P = 128         # partitions
CH = 512        # q-chunk size
XC = 1024       # x-stream DMA chunk
VW = 128        # padded v-stationary width per head: [v(64) | ones | zeros]
LAG = 2         # attn@v trails scores by this many k-tiles


def build(nc, tile, mybir, s=S, d=D):
    F32R = mybir.dt.float32r
    F32 = mybir.dt.float32
    BF16 = mybir.dt.bfloat16
    Exp = mybir.ActivationFunctionType.Exp
    xc = min(XC, s)    # x DMA chunk
    nch = s // CH      # q-chunks
    nst = s // P       # s-tiles (also k-tiles)
    nd = d // P        # d-tiles
    nxc = s // xc      # x DMA chunks
    nm = HDC // P      # head-pair tiles
    spx = xc // CH     # q-chunks per x chunk

    xqT = nc.dram_tensor("xqT", [d, s], BF16, kind="ExternalInput").ap()
    xkT = nc.dram_tensor("xkT", [d, s], BF16, kind="ExternalInput").ap()
    xvT = nc.dram_tensor("xvT", [d, s], BF16, kind="ExternalInput").ap()
    wqkv = nc.dram_tensor("wqkv", [d, 3 * HDC], BF16, kind="ExternalInput").ap()
    wo = nc.dram_tensor("wo", [HDC, d], BF16, kind="ExternalInput").ap()
    maskA = nc.dram_tensor("maskA", [P, P], BF16, kind="ExternalInput").ap()
    vinit = nc.dram_tensor("vinit", [P, nst * HPC * VW], BF16,
                           kind="ExternalInput").ap()
    onesf = nc.dram_tensor("onesf", [1, P], F32R, kind="ExternalInput").ap()
    out = nc.dram_tensor("out", [s, d], BF16, kind="ExternalOutput").ap()

    with tile.TileContext(nc) as tc:
        from contextlib import ExitStack
        with ExitStack() as ctx:
            wp = ctx.enter_context(tc.tile_pool(name="wp", bufs=1))
            xp = ctx.enter_context(tc.tile_pool(name="xp", bufs=6))
            per = ctx.enter_context(tc.tile_pool(name="per", bufs=1))
            ep = ctx.enter_context(tc.tile_pool(name="ep", bufs=6))
            sp = ctx.enter_context(tc.tile_pool(name="sp", bufs=2))
            obp = ctx.enter_context(tc.tile_pool(name="obp", bufs=3))
            scp = ctx.enter_context(tc.tile_pool(name="scp", bufs=2, space="PSUM"))
            ovp = ctx.enter_context(tc.tile_pool(name="ovp", bufs=4, space="PSUM"))

            # --- persistent tiles ---
            wqkv_t = [wp.tile([P, 3 * HDC], BF16, name=f"wqkv{i}")
                      for i in range(nd)]
            wq_t = [wqkv_t[i][:, 0:HDC] for i in range(nd)]
            wk_t = [wqkv_t[i][:, HDC:2 * HDC] for i in range(nd)]
            wv_t = [wqkv_t[i][:, 2 * HDC:3 * HDC] for i in range(nd)]
            wo_t = [wp.tile([P, d], BF16, name=f"wo{i}") for i in range(nm)]
            mA = wp.tile([P, P], BF16, name="mA")
            onf = wp.tile([1, P], F32R, name="onf")
            qT = [per.tile([P, s], BF16, name=f"qT{m}") for m in range(nm)]
            kTt = [per.tile([P, s], BF16, name=f"kT{m}") for m in range(nm)]
            oT = [per.tile([P, s], BF16, name=f"oT{m}") for m in range(nm)]
            vaug = per.tile([P, nst * HPC * VW], BF16, name="vaug")
            den = per.tile([P, CH], F32, name="den")
            rec = per.tile([P, CH], F32R, name="rec")

            # --- x loads: one DMA per (stream, 512-col half) covers all 8
            # d-tiles (contiguous DRAM rows -> strided src AP), so the three
            # streams load in parallel on the three DMA queues ---
            xt = {}  # (stream, chunk) -> [P, nd*CH] tile, d-tiles side by side
            def issue_xh(stream, xsrc, c, eng):
                t = xp.tile([P, nd * CH], BF16, name="xt", tag="xt")
                xt[(stream, c)] = t
                hd = nd // 2
                for g in range(2):  # split by d-halves so matmuls can start
                    eng.dma_start(
                        t[:, g * hd * CH:(g + 1) * hd * CH].rearrange(
                            "p (dd x) -> p dd x", x=CH),
                        xsrc[g * hd * P:(g + 1) * hd * P,
                             c * CH:(c + 1) * CH].rearrange(
                            "(dd p) x -> p dd x", p=P))

            q3 = (nc.sync, nc.gpsimd, nc.scalar)
            q2 = (nc.sync, nc.gpsimd)
            issue_xh(2, xqT, 0, nc.sync)
            issue_xh(1, xkT, 0, nc.gpsimd)
            nc.scalar.dma_start(mA[:], maskA[:, :])
            for i in range(nd):
                q3[i % 3].dma_start(wqkv_t[i][:], wqkv[i * P:(i + 1) * P, :])
            issue_xh(0, xvT, 0, nc.scalar)
            issue_xh(2, xqT, 1, nc.sync)
            issue_xh(1, xkT, 1, nc.gpsimd)
            issue_xh(0, xvT, 1, nc.scalar)
            nc.scalar.dma_start(onf[:], onesf[:, :])
            for i in range(nm):
                nc.scalar.dma_start(wo_t[i][:], wo[i * P:(i + 1) * P, :])
            half = nst * HPC * VW // 2
            nc.sync.dma_start(vaug[:, 0:half], vinit[:, 0:half])
            nc.gpsimd.dma_start(vaug[:, half:], vinit[:, half:])

            def vproject(c):
                """vaug[st-block of chunk c] = v natural [s_k, dv] per head."""
                xh = xt[(0, c)]
                pp = scp.tile([P, 2 * CH], F32, name="pv", tag="sc")
                for j in range(CH // P):
                    for dd in range(nd):
                        nc.tensor.matmul(
                            pp[:, j * HDC:(j + 1) * HDC],
                            xh[:, dd * CH + j * P:dd * CH + (j + 1) * P],
                            wv_t[dd][:],
                            start=(dd == 0), stop=(dd == nd - 1))
                for j in range(CH // P):
                    st = c * (CH // P) + j
                    base = st * HPC * VW
                    dst = vaug[:, base:base + HPC * VW].rearrange(
                        "p (h x) -> p h x", x=VW)[:, :, 0:DV]
                    src = pp[:, j * HDC:(j + 1) * HDC].rearrange(
                        "p (h x) -> p h x", x=DV)
                    nc.vector.tensor_copy(dst, src)

            def project(stream, w_t, dstT, c):
                """dstT[m][:, c*CH:(c+1)*CH] for both m from one psum tile."""
                xh = xt[(stream, c)]
                pp = scp.tile([P, 2 * CH], F32, name="pbig", tag="sc")
                for m in range(nm):
                    for dd in range(nd):
                        nc.tensor.matmul(
                            pp[:, m * CH:(m + 1) * CH],
                            w_t[dd][:, m * P:(m + 1) * P],
                            xh[:, dd * CH:(dd + 1) * CH],
                            start=(dd == 0), stop=(dd == nd - 1))
                for m in range(nm):
                    dsl = dstT[m][:, c * CH:(c + 1) * CH]
                    if m % 2 == 0:
                        nc.scalar.copy(dsl, pp[:, m * CH:(m + 1) * CH])
                    else:
                        nc.vector.tensor_copy(dsl, pp[:, m * CH:(m + 1) * CH])

            def attention_pair(hp, c):
                """Heads hp, hp+1 (one m-tile) over chunk c, ov lagged."""
                mi = hp // 2
                nt = 4 * c + 4  # k-tiles for this chunk
                ov = [ovp.tile([P, CH], F32, name=f"ov{j}", tag="ov")
                      for j in range(2)]
                pend = []

                def emit_ov(t, ex, lo):
                    for j in range(2):
                        vb = t * HPC * VW + (hp + j) * VW
                        nc.tensor.matmul(
                            ov[j][:, lo:CH],
                            vaug[:, vb:vb + VW],
                            ex[:, j * CH + lo:(j + 1) * CH],
                            start=(t == 0), stop=(t == nt - 1))

                for t in range(nt):
                    r = t - 4 * c
                    lo = max(r, 0) * P  # first valid column in the chunk
                    sc_t = scp.tile([P, 2 * CH], F32, name="scp", tag="sc")
                    for j in range(2):
                        nc.tensor.matmul(
                            sc_t[:, j * CH + lo:(j + 1) * CH],
                            kTt[mi][j * DK:(j + 1) * DK, t * P:(t + 1) * P],
                            qT[mi][j * DK:(j + 1) * DK,
                                   c * CH + lo:(c + 1) * CH],
                            start=True, stop=True)
                    ex = ep.tile([P, 2 * CH], BF16, name="ex", tag="ex")
                    sview = sc_t[:].rearrange("p (g x) -> p g x", x=CH)
                    eview = ex[:].rearrange("p (g x) -> p g x", x=CH)
                    nc.scalar.activation(eview[:, :, lo:CH],
                                         sview[:, :, lo:CH], Exp)
                    if r >= 0:
                        for j in range(2):
                            nc.vector.tensor_mul(
                                ex[:, j * CH + lo:j * CH + lo + P],
                                ex[:, j * CH + lo:j * CH + lo + P], mA[:])
                    pend.append((t, ex, lo))
                    if len(pend) > LAG:
                        emit_ov(*pend.pop(0))
                while pend:
                    emit_ov(*pend.pop(0))
                # numerator rows 0:64 -> oT (unnormalized, bf16);
                # denominator row 64 -> denw col 4c+h (partition-spread)
                tail_pair = c == nch - 1 and hp == 2
                for j in range(2):
                    h, ri = hp + j, j * DK
                    dsl = oT[mi][ri:ri + DK, c * CH:(c + 1) * CH]
                    dstg = sp.tile([1, CH], F32, name="dstg", tag="dstg", bufs=4)
                    if tail_pair:
                        nc.scalar.copy(dsl, ov[j][0:DV, :])
                        nc.scalar.copy(dstg[:], ov[j][DV:DV + 1, :])
                    else:
                        nc.vector.tensor_copy(dsl, ov[j][0:DV, :])
                        nc.vector.tensor_copy(dstg[:], ov[j][DV:DV + 1, :])
                    nc.scalar.dma_start(den[32 * c + h:32 * c + h + 1, :],
                                         dstg[:])

            def normalize_pair(c, hp):
                """Scale oT rows of heads hp,hp+1 of chunk c by 1/denominator."""
                mi = hp // 2
                last = c == nch - 1
                def recip(n_):
                    r0 = 32 * c
                    with nc.allow_low_precision(reason="softmax denom recip"):
                        nc.vector.reciprocal(rec[r0:r0 + n_, :],
                                             den[r0:r0 + n_, :])
                if hp == 0 and not last:
                    recip(HPC)
                if last:
                    recip(2 if hp == 0 else HPC)
                rb = scp.tile([P, 2 * CH], F32, name="rb", tag="sc")
                for j in range(2):
                    h = hp + j
                    stg = sp.tile([1, CH], F32R, name="stg", tag="stg", bufs=4)
                    nc.sync.dma_start(stg[:],
                                      rec[32 * c + h:32 * c + h + 1, :])
                    nc.tensor.matmul(rb[:, j * CH:(j + 1) * CH],
                                     onf[0:1, :], stg[:],
                                     start=True, stop=True)
                if last and hp == 2:
                    for j in range(2):
                        ri = j * DK
                        sl = oT[mi][ri:ri + DK, c * CH:(c + 1) * CH]
                        nc.vector.tensor_mul(
                            sl, sl, rb[ri:ri + DK, j * CH:(j + 1) * CH])
                else:
                    recT = sp.tile([P, 2 * CH], BF16, name="recT", tag="recT",
                                   bufs=2)
                    nc.vector.tensor_copy(recT[:], rb[:])
                    for j in range(2):
                        ri = j * DK
                        sl = oT[mi][ri:ri + DK, c * CH:(c + 1) * CH]
                        nc.gpsimd.tensor_mul(
                            sl, sl, recT[ri:ri + DK, j * CH:(j + 1) * CH])

            def oproj(st):
                pp = scp.tile([P, 2 * CH], F32, name="pout", tag="sc")
                for n in range(d // 512):
                    for m in range(nm):
                        nc.tensor.matmul(pp[:, n * 512:(n + 1) * 512],
                                         oT[m][:, st * P:(st + 1) * P],
                                         wo_t[m][:, n * 512:(n + 1) * 512],
                                         start=(m == 0), stop=(m == nm - 1))
                ob = obp.tile([P, d], BF16, name="ob", tag="ob")
                if st % 2 == 0:
                    nc.scalar.copy(ob[:], pp[:])
                else:
                    nc.vector.tensor_copy(ob[:], pp[:])
                nc.sync.dma_start(out[st * P:(st + 1) * P, :], ob[:])

            # --- per-chunk pipeline ---
            def proj_chunk(c):
                project(2, wq_t, qT, c)
                project(1, wk_t, kTt, c)
                vproject(c)

            prev = None
            proj_chunk(0)
            for c in range(nch):
                attention_pair(0, c)
                if c == 1:
                    issue_xh(2, xqT, 2, nc.sync)
                    issue_xh(1, xkT, 2, nc.gpsimd)
                if prev is not None:
                    normalize_pair(prev, 0)
                    normalize_pair(prev, 2)
                attention_pair(2, c)
                if c == 1:
                    issue_xh(0, xvT, 2, nc.sync)
                    issue_xh(2, xqT, 3, nc.sync)
                    issue_xh(1, xkT, 3, nc.gpsimd)
                    issue_xh(0, xvT, 3, nc.gpsimd)
                if c + 1 < nch:
                    proj_chunk(c + 1)
                if c == nch - 1:
                    normalize_pair(c, 0)
                if prev is not None:
                    for st in range(prev * CH // P, (prev + 1) * CH // P):
                        oproj(st)
                prev = c
            normalize_pair(prev, 2)
            for st in range(prev * CH // P, (prev + 1) * CH // P):
                oproj(st)
    nc.compile()
    return nc


_NC_CACHE = {}
LAST_RESULT = None


def _get_nc(s=S, d=D):
    key = (s, d)
    if key not in _NC_CACHE:
        import concourse.tile as tile
        import concourse.mybir as mybir
        from concourse import bacc
        nc = bacc.Bacc("TRN2", target_bir_lowering=False, num_devices=NCORES)
        _NC_CACHE[key] = build(nc, tile, mybir, s=s, d=d)
    return _NC_CACHE[key]


def make_consts():
    import ml_dtypes
    i = np.arange(P)[:, None]
    j = np.arange(P)[None, :]
    maskA = (j >= i).astype(ml_dtypes.bfloat16)
    nst = S // P
    vinit = np.zeros((P, nst * HPC * VW), dtype=ml_dtypes.bfloat16)
    vinit[:, DV::VW] = 1
    onesf = np.ones((1, P), dtype=np.float32)
    return maskA, vinit, onesf


def kernel(Q, K, V, Wq, Wk, Wv, Wo):
    import ml_dtypes
    from concourse.bass_utils import run_bass_kernel_spmd

    BF = ml_dtypes.bfloat16
    Q = np.asarray(Q, dtype=np.float32)
    K = np.asarray(K, dtype=np.float32)
    V = np.asarray(V, dtype=np.float32)
    Wq = np.asarray(Wq, dtype=np.float32) * np.float32(1.0 / np.sqrt(DK))
    Wk = np.asarray(Wk, dtype=np.float32)
    Wv = np.asarray(Wv, dtype=np.float32)
    Wo = np.asarray(Wo, dtype=np.float32)

    QT = [np.ascontiguousarray(Q[b].T).astype(BF) for b in range(B)]
    KT = [np.ascontiguousarray(K[b].T).astype(BF) for b in range(B)]
    VT = [np.ascontiguousarray(V[b].T).astype(BF) for b in range(B)]
    maskA, vinit, onesf = make_consts()

    in_maps = []
    for core in range(NCORES):
        b, g = core // HG, core % HG
        cs = slice(g * HDC, (g + 1) * HDC)
        in_maps.append({
            "xqT": QT[b], "xkT": KT[b], "xvT": VT[b],
            "wqkv": np.ascontiguousarray(
                np.concatenate([Wq[:, cs], Wk[:, cs], Wv[:, cs]],
                               axis=1)).astype(BF),
            "wo": np.ascontiguousarray(Wo[cs, :]).astype(BF),
            "maskA": maskA, "vinit": vinit, "onesf": onesf,
        })

    nc = _get_nc()
    res = run_bass_kernel_spmd(nc, in_maps, core_ids=list(range(NCORES)))
    global LAST_RESULT
    LAST_RESULT = res

    acc = np.zeros((B, S, D), dtype=np.float64)
    for core in range(NCORES):
        acc[core // HG] += res.results[core]["out"].astype(np.float64)
    return acc.astype(np.float32)
